# revision 1
# baseline (speedup 1.0000x reference)
"""Multi-head attention (B=2, N=2048, D=1024, H=16, dh=64) on 8 TRN2 cores.

Sharding: tensor-parallel over heads -- 2 heads per core. Each core computes
its heads' Q/K/V projections, attention, and a partial output projection
(rows of Wo for its heads); the host sums the 8 partial outputs.

Per-core layout strategy:
  - Host supplies X^T ([B, D, N], bf16) so projections contract D with W as
    the stationary operand, producing Q^T/K^T ([local_dim, tok]) directly.
  - scoresT[k, q] = (K^T slice).T @ Q^T slice per head (dh=64 contraction).
  - softmax without max subtraction (scores ~ N(0,1); exp is fp32-safe).
  - ctxT[dv, q] accumulated over k-chunks with V augmented by a ones column,
    yielding the softmax denominator Z as row 64 for free.
  - normalize: recip(Z) -> partition_broadcast -> multiply ctxT.
  - output projection: out[q, od] = stacked-ctxT.T @ Wo_local, written as
    fp32 partials; host sums partials and adds bo.
"""

import numpy as np
import ml_dtypes
from contextlib import ExitStack

import concourse.bass as bass
import concourse.tile as tile
from concourse import bacc, mybir
from concourse.bass import ts, ds
from concourse.bass_utils import run_bass_kernel_spmd
from concourse.masks import make_identity
from concourse import library_config

BF16 = mybir.dt.bfloat16
F32 = mybir.dt.float32

B = 2
N = 2048          # tokens per batch
D = 1024          # model dim
NCORES = 8
DLOC = 128        # local dims per core (2 heads x 64)
DH = 64
QS = 512          # q slice
NQS = N // QS     # 4 per batch
NKT = N // 128    # 16 k-tiles of 128
NDCH = D // 128   # 8 d-chunks


def _build_program():
    nc = bacc.Bacc("TRN2", target_bir_lowering=False, debug=False)

    xT = {}
    w = {}
    bias = {}
    for t in ("q", "k", "v"):
        xT[t] = nc.dram_tensor(f"x{t}T", [B, D, N], BF16, kind="ExternalInput").ap()
        w[t] = nc.dram_tensor(f"w{t}", [D, DLOC], BF16, kind="ExternalInput").ap()
        bias[t] = nc.dram_tensor(f"b{t}", [DLOC, 1], F32, kind="ExternalInput").ap()
    wo = nc.dram_tensor("wo", [DLOC, D], BF16, kind="ExternalInput").ap()
    outp = nc.dram_tensor("outp", [B * N, D], F32, kind="ExternalOutput").ap()
    zscr = nc.dram_tensor("zscr", [B * NQS, 2 * QS], F32).ap()

    with ExitStack() as ctx:
        tc = ctx.enter_context(tile.TileContext(nc))

        const = ctx.enter_context(tc.tile_pool(name="const", bufs=1))
        xpool = ctx.enter_context(tc.tile_pool(name="xchunks", bufs=10))
        qkpool = ctx.enter_context(tc.tile_pool(name="qk", bufs=4))
        vtpool = ctx.enter_context(tc.tile_pool(name="vt", bufs=2))
        vaugp = ctx.enter_context(tc.tile_pool(name="vaug", bufs=4))
        expp = ctx.enter_context(tc.tile_pool(name="expT", bufs=6))
        zpool = ctx.enter_context(tc.tile_pool(name="zr", bufs=2))
        bcpool = ctx.enter_context(tc.tile_pool(name="bc", bufs=4))
        stackp = ctx.enter_context(tc.tile_pool(name="stack", bufs=2))
        hbufp = ctx.enter_context(tc.tile_pool(name="hbuf", bufs=2))
        outsb = ctx.enter_context(tc.tile_pool(name="outsb", bufs=3))

        pp_shared = ctx.enter_context(tc.tile_pool(name="pp_shared", bufs=2, space="PSUM"))
        pp_sc = ctx.enter_context(tc.tile_pool(name="pp_sc", bufs=2, space="PSUM"))
        pp_ctx = ctx.enter_context(tc.tile_pool(name="pp_ctx", bufs=4, space="PSUM"))

        # ---- constants ----
        ident = const.tile([128, 128], BF16, tag="ident")
        make_identity(nc, ident)
        w_sb = {}
        b_sb = {}
        for t in ("q", "k", "v"):
            w_sb[t] = const.tile([128, NDCH, DLOC], BF16, tag=f"w{t}", name=f"w{t}sb")
            nc.sync.dma_start(out=w_sb[t], in_=w[t].rearrange("(c p) m -> p c m", p=128))
            b_sb[t] = const.tile([128, 1], F32, tag=f"b{t}", name=f"b{t}sb")
            nc.sync.dma_start(out=b_sb[t], in_=bias[t])
        wo_sb = const.tile([128, D], BF16, tag="wo")
        nc.sync.dma_start(out=wo_sb, in_=wo)

        # ---- projections ----
        qt_sb = {}   # [b] -> [128, N] bf16  (Q^T, local dims on partitions)
        kt_sb = {}
        vaug = {}    # [b][h] -> [128, NKT, 65] bf16 (V chunks + ones col)

        for b in range(B):
            xtiles = {}
            for t in ("q", "k", "v"):
                for c in range(NDCH):
                    xt_ = xpool.tile([128, N], BF16, tag="x", name="xt")
                    nc.sync.dma_start(out=xt_, in_=xT[t][b, ts(c, 128), :])
                    xtiles[(t, c)] = xt_

            for t in ("q", "k", "v"):
                if t == "v":
                    tgt = vtpool.tile([128, N], BF16, tag="vt", name="vt_t")
                else:
                    tgt = qkpool.tile([128, N], BF16, tag="qk", name="qk_t")
                for s in range(NQS):
                    ps = pp_shared.tile([128, QS], F32, tag="shared", name="ps_proj")
                    for c in range(NDCH):
                        nc.tensor.matmul(
                            ps,
                            lhsT=w_sb[t][:, c, :],
                            rhs=xtiles[(t, c)][:, ts(s, QS)],
                            start=(c == 0),
                            stop=(c == NDCH - 1),
                        )
                    nc.vector.tensor_scalar_add(tgt[:, ts(s, QS)], ps, b_sb[t])
                if t == "q":
                    qt_sb[b] = tgt
                elif t == "k":
                    kt_sb[b] = tgt
                else:
                    # transpose V^T -> V (tokens on partitions), split heads,
                    # append ones column for the softmax denominator.
                    vaug[b] = {}
                    for h in range(2):
                        va = vaugp.tile([128, NKT, 65], BF16, tag="vaug", name="va_t")
                        nc.vector.memset(va[:, :, 64:65], 1.0)
                        vaug[b][h] = va
                    for tk in range(NKT):
                        pt = pp_shared.tile([128, 128], BF16, tag="shared", name="pt_tr")
                        nc.tensor.transpose(pt, tgt[:, ts(tk, 128)], ident)
                        nc.vector.tensor_copy(out=vaug[b][0][:, tk, 0:64], in_=pt[:, 0:64])
                        nc.vector.tensor_copy(out=vaug[b][1][:, tk, 0:64], in_=pt[:, 64:128])

        # ---- attention (wo-phase deferred by one q-slice for overlap) ----
        pending = None  # (stack_tile, b, qs)

        def emit_wo(stack_t, b_, qs_):
            for qsub in range(QS // 128):
                for od in range(D // QS):
                    pw = pp_shared.tile([128, QS], F32, tag="shared", name="pw_wo")
                    nc.tensor.matmul(
                        pw,
                        lhsT=stack_t[:, ts(qsub, 128)],
                        rhs=wo_sb[:, ts(od, QS)],
                        start=True, stop=True,
                    )
                    ob = outsb.tile([128, QS], F32, tag="out", name="ob_out")
                    nc.vector.tensor_copy(out=ob, in_=pw)
                    row0 = b_ * N + qs_ * QS + qsub * 128
                    nc.sync.dma_start(out=outp[ds(row0, 128), ts(od, QS)], in_=ob)

        for b in range(B):
            for qs in range(NQS):
                psC = []
                for h in range(2):
                    psC.append(pp_ctx.tile([128, QS], F32, tag="ctx", name="ps_ctx"))
                for kt in range(NKT):
                    for h in range(2):
                        lo, hi = (0, 64) if h == 0 else (64, 128)
                        psS = pp_sc.tile([128, QS], F32, tag="sc", name="ps_sc")
                        nc.tensor.matmul(
                            psS,
                            lhsT=kt_sb[b][lo:hi, ts(kt, 128)],
                            rhs=qt_sb[b][lo:hi, ts(qs, QS)],
                            start=True, stop=True,
                        )
                        e = expp.tile([128, QS], BF16, tag="expT", name="e_t")
                        nc.scalar.activation(e, psS, mybir.ActivationFunctionType.Exp)
                        nc.tensor.matmul(
                            psC[h][0:65, :],
                            lhsT=vaug[b][h][:, kt, :],
                            rhs=e,
                            start=(kt == 0),
                            stop=(kt == NKT - 1),
                        )

                # normalization chain
                zr = zpool.tile([128, 2 * QS], F32, tag="zr")
                nc.vector.reciprocal(out=zr[64:65, 0:QS], in_=psC[0][64:65, :])
                nc.vector.reciprocal(out=zr[64:65, QS:2 * QS], in_=psC[1][64:65, :])
                bc0 = bcpool.tile([128, QS], F32, tag="bc", name="bc0")
                bc1 = bcpool.tile([128, QS], F32, tag="bc", name="bc1")
                # partition-broadcast via DRAM roundtrip (engines cannot
                # replicate across partitions; DRAM-source DMA can).
                zrow = zscr[b * NQS + qs, :]
                nc.sync.dma_start(out=zrow, in_=zr[64:65, :])
                for h, bc in ((0, bc0), (1, bc1)):
                    seg = zscr[b * NQS + qs, ds(h * QS, QS)]
                    nc.sync.dma_start(
                        out=bc[0:64, :],
                        in_=bass.AP(tensor=seg.tensor, offset=seg.offset,
                                    ap=[[0, 64]] + list(seg.ap)))
                stack_t = stackp.tile([128, QS], BF16, tag="stack")
                hb = hbufp.tile([128, QS], BF16, tag="hbuf")
                nc.vector.tensor_mul(stack_t[0:64, :], psC[0][0:64, :], bc0[0:64, :])
                nc.vector.tensor_mul(hb[0:64, :], psC[1][0:64, :], bc1[0:64, :])
                nc.sync.dma_start(out=stack_t[64:128, :], in_=hb[0:64, :])

                if pending is not None:
                    emit_wo(*pending)
                pending = (stack_t, b, qs)

        emit_wo(*pending)

    nc.compile()
    return nc


_NC = None


def _get_nc():
    global _NC
    if _NC is None:
        _NC = _build_program()
    return _NC


def _host_prep(query, key, value, Wq, bq, Wk, bk, Wv, bv, Wo, bo):
    bf16 = ml_dtypes.bfloat16
    f32 = np.float32
    q = np.asarray(query, f32)
    k = np.asarray(key, f32)
    v = np.asarray(value, f32)
    Wq = np.asarray(Wq, f32)
    Wk = np.asarray(Wk, f32)
    Wv = np.asarray(Wv, f32)
    Wo = np.asarray(Wo, f32)
    bq = np.asarray(bq, f32)
    bk = np.asarray(bk, f32)
    bv = np.asarray(bv, f32)

    scale = 1.0 / np.sqrt(DH).astype(f32)
    xqT = np.ascontiguousarray(q.transpose(0, 2, 1)).astype(bf16)
    xkT = np.ascontiguousarray(k.transpose(0, 2, 1)).astype(bf16)
    xvT = np.ascontiguousarray(v.transpose(0, 2, 1)).astype(bf16)

    in_maps = []
    for c in range(NCORES):
        sl = slice(c * DLOC, (c + 1) * DLOC)
        in_maps.append({
            "xqT": xqT, "xkT": xkT, "xvT": xvT,
            "wq": np.ascontiguousarray(Wq[:, sl] * scale).astype(bf16),
            "wk": np.ascontiguousarray(Wk[:, sl]).astype(bf16),
            "wv": np.ascontiguousarray(Wv[:, sl]).astype(bf16),
            "bq": np.ascontiguousarray((bq[sl] * scale).reshape(DLOC, 1)),
            "bk": np.ascontiguousarray(bk[sl].reshape(DLOC, 1)),
            "bv": np.ascontiguousarray(bv[sl].reshape(DLOC, 1)),
            "wo": np.ascontiguousarray(Wo[sl, :]).astype(bf16),
        })
    return in_maps


def _run(in_maps, trace=False):
    nc = _get_nc()
    return run_bass_kernel_spmd(nc, in_maps, list(range(NCORES)), trace=trace)


def kernel(query, key, value, Wq, bq, Wk, bk, Wv, bv, Wo, bo):
    in_maps = _host_prep(query, key, value, Wq, bq, Wk, bk, Wv, bv, Wo, bo)
    res = _run(in_maps)
    acc = np.zeros((B * N, D), np.float32)
    for c in range(NCORES):
        acc += res.results[c]["outp"]
    acc += np.asarray(bo, np.float32)[None, :]
    return acc.reshape(B, N, D)



# revision 8
# speedup vs baseline: 1.4272x; 1.4272x over previous
"""Multi-head attention (B=2, N=2048, D=1024, H=16, dh=64) on 8 TRN2 cores.

Sharding: tensor-parallel over heads -- 2 heads per core. Each core computes
its heads' Q/K/V projections, attention, and a partial output projection
(rows of Wo for its heads); the host sums the 8 partial outputs (bf16).

Per-core layout strategy (v2 -- engine-balanced, HAM-warm):
  - Projections contract D with W stationary, producing Q^T/K^T/V^T
    ([local_dim, tok]); bias-adds run on the Scalar engine (idle then).
  - Scores: per k-tile, the two heads' [128,512] score matmuls are
    row-tiled (K=64 at array rows 0-63 / 64-127) into one [128,1024]
    two-bank PSUM tile; ONE exp activation covers both heads.
  - V is transposed via PE and packed into one vaug tile per batch:
    [V0 | 1 | V1 | 0 | 1] so head0's ctx matmul (M=65) puts Z0 at PSUM
    partition 64 and head1's (M=66) puts Z1 at partition 65 -- the two
    denominator rows land on distinct partitions and need only ONE
    reciprocal per q-slice.
  - ctx rows are evacuated to SBUF immediately (frees PSUM), normalized
    by a DRAM-roundtrip broadcast of 1/Z, and fed to the Wo matmul.
  - Partial outputs are written bf16 (halves writeback DMA).
"""

import numpy as np
import ml_dtypes
from contextlib import ExitStack

import concourse.bass as bass
import concourse.tile as tile
from concourse import bacc, mybir
from concourse.bass import ts, ds
from concourse.bass_utils import run_bass_kernel_spmd
from concourse.masks import make_identity

BF16 = mybir.dt.bfloat16
F32 = mybir.dt.float32

B = 2
N = 2048          # tokens per batch
D = 1024          # model dim
NCORES = 8
DLOC = 128        # local dims per core (2 heads x 64)
DH = 64
QS = 512          # q slice
NQS = N // QS     # 4 per batch
NKT = N // 128    # 16 k-tiles of 128
NDCH = D // 128   # 8 d-chunks
VW = 131          # vaug width: V0(64) | zero | ones | V1(64) | ones


def _build_program():
    nc = bacc.Bacc("TRN2", target_bir_lowering=False, debug=False)

    xT = {}
    w = {}
    bias = {}
    for t in ("q", "k", "v"):
        xT[t] = nc.dram_tensor(f"x{t}T", [B, D, N], BF16, kind="ExternalInput").ap()
        w[t] = nc.dram_tensor(f"w{t}", [D, DLOC], BF16, kind="ExternalInput").ap()
        bias[t] = nc.dram_tensor(f"b{t}", [DLOC, 1], F32, kind="ExternalInput").ap()
    wo = nc.dram_tensor("wo", [DLOC, D], BF16, kind="ExternalInput").ap()
    outp = nc.dram_tensor("outp", [B * N, D], BF16, kind="ExternalOutput").ap()
    zscr = nc.dram_tensor("zscr", [B * NQS, 2 * QS], F32).ap()

    with ExitStack() as ctx:
        tc = ctx.enter_context(tile.TileContext(nc))

        const = ctx.enter_context(tc.tile_pool(name="const", bufs=1))
        xpool = ctx.enter_context(tc.tile_pool(name="xchunks", bufs=16))
        qkpool = ctx.enter_context(tc.tile_pool(name="qk", bufs=4))
        vtpool = ctx.enter_context(tc.tile_pool(name="vt", bufs=2))
        vaugp = ctx.enter_context(tc.tile_pool(name="vaug", bufs=2))
        expp = ctx.enter_context(tc.tile_pool(name="expT", bufs=3))
        zpool = ctx.enter_context(tc.tile_pool(name="zr", bufs=2))
        bcpool = ctx.enter_context(tc.tile_pool(name="bc", bufs=4))
        csbp = ctx.enter_context(tc.tile_pool(name="csb", bufs=4))
        stackp = ctx.enter_context(tc.tile_pool(name="stack", bufs=2))
        hbufp = ctx.enter_context(tc.tile_pool(name="hbuf", bufs=2))
        outsb = ctx.enter_context(tc.tile_pool(name="outsb", bufs=3))

        pp_shared = ctx.enter_context(tc.tile_pool(name="pp_shared", bufs=2, space="PSUM"))
        pp_sc = ctx.enter_context(tc.tile_pool(name="pp_sc", bufs=2, space="PSUM"))
        pp_ctx = ctx.enter_context(tc.tile_pool(name="pp_ctx", bufs=2, space="PSUM"))

        # ---- constants ----
        ident = const.tile([128, 128], BF16, tag="ident")
        make_identity(nc, ident)
        w_sb = {}
        b_sb = {}
        for t in ("q", "k", "v"):
            w_sb[t] = const.tile([128, NDCH, DLOC], BF16, tag=f"w{t}", name=f"w{t}sb")
            nc.sync.dma_start(out=w_sb[t], in_=w[t].rearrange("(c p) m -> p c m", p=128))
            b_sb[t] = const.tile([128, 1], F32, tag=f"b{t}", name=f"b{t}sb")
            nc.sync.dma_start(out=b_sb[t], in_=bias[t])
        wo_sb = const.tile([128, D], BF16, tag="wo")
        nc.sync.dma_start(out=wo_sb, in_=wo)

        # ---- projections ----
        qt_sb = {}   # [b] -> [128, N] bf16  (Q^T, local dims on partitions)
        kt_sb = {}
        vaug = {}    # [b] -> [128, NKT, VW] bf16

        for b in range(B):
            xtiles = {}
            for t in ("q", "k", "v"):
                for c in range(NDCH):
                    xt_ = xpool.tile([128, N], BF16, tag="x", name="xt")
                    nc.sync.dma_start(out=xt_, in_=xT[t][b, ts(c, 128), :])
                    xtiles[(t, c)] = xt_

            for t in ("q", "k", "v"):
                if t == "v":
                    tgt = vtpool.tile([128, N], BF16, tag="vt", name="vt_t")
                else:
                    tgt = qkpool.tile([128, N], BF16, tag="qk", name="qk_t")
                for s in range(NQS):
                    ps = pp_shared.tile([128, QS], F32, tag="shared", name="ps_proj")
                    for c in range(NDCH):
                        nc.tensor.matmul(
                            ps,
                            lhsT=w_sb[t][:, c, :],
                            rhs=xtiles[(t, c)][:, ts(s, QS)],
                            start=(c == 0),
                            stop=(c == NDCH - 1),
                        )
                    # bias-add on the Scalar engine (idle during projections)
                    nc.scalar.activation(
                        tgt[:, ts(s, QS)], ps,
                        mybir.ActivationFunctionType.Identity,
                        bias=b_sb[t],
                    )
                if t == "q":
                    qt_sb[b] = tgt
                elif t == "k":
                    kt_sb[b] = tgt
                else:
                    # transpose V^T -> V (tokens on partitions), pack both
                    # heads + denominator columns into one vaug tile.
                    va = vaugp.tile([128, NKT, VW], BF16, tag="vaug", name="va_t")
                    nc.vector.memset(va[:, :, 64:65], 0.0)
                    nc.vector.memset(va[:, :, 65:66], 1.0)
                    nc.vector.memset(va[:, :, 130:131], 1.0)
                    vaug[b] = va
                    for tk in range(NKT):
                        pt = pp_shared.tile([128, 128], BF16, tag="shared", name="pt_tr")
                        nc.tensor.transpose(pt, tgt[:, ts(tk, 128)], ident)
                        # one strided copy: head h -> cols [66h, 66h+64)
                        dst = bass.AP(
                            tensor=va.tensor,
                            offset=va.offset + tk * VW,
                            ap=[list(va.ap[0]), [66, 2], [1, 64]],
                        )
                        src = bass.AP(
                            tensor=pt.tensor,
                            offset=pt.offset,
                            ap=[list(pt.ap[0]), [64, 2], [1, 64]],
                        )
                        nc.vector.tensor_copy(out=dst, in_=src)

        # ---- attention (wo-phase deferred by one q-slice for overlap) ----
        pending = None  # (stack_tile, b, qs)

        def emit_wo(stack_t, b_, qs_):
            for qsub in range(QS // 128):
                for od in range(D // QS):
                    pw = pp_shared.tile([128, QS], F32, tag="shared", name="pw_wo")
                    nc.tensor.matmul(
                        pw,
                        lhsT=stack_t[:, ts(qsub, 128)],
                        rhs=wo_sb[:, ts(od, QS)],
                        start=True, stop=True,
                    )
                    ob = outsb.tile([128, QS], BF16, tag="out", name="ob_out")
                    nc.vector.tensor_copy(out=ob, in_=pw)
                    row0 = b_ * N + qs_ * QS + qsub * 128
                    nc.sync.dma_start(out=outp[ds(row0, 128), ts(od, QS)], in_=ob)

        for b in range(B):
            for qs in range(NQS):
                # psC0 rows: 0-63 ctx0, 64 zero, 65 Z0; psC1: 0-63 ctx1, 64 Z1
                psC0 = pp_ctx.tile([66, QS], F32, tag="ctx", name="ps_ctx0")
                psC1 = pp_ctx.tile([65, QS], F32, tag="ctx", name="ps_ctx1")
                for kt in range(NKT):
                    psS = pp_sc.tile([128, 2 * QS], F32, tag="sc", name="ps_sc")
                    nc.tensor.matmul(
                        psS[:, 0:QS],
                        lhsT=kt_sb[b][0:64, ts(kt, 128)],
                        rhs=qt_sb[b][0:64, ts(qs, QS)],
                        start=True, stop=True,
                    )
                    nc.tensor.matmul(
                        psS[:, QS:2 * QS],
                        lhsT=kt_sb[b][64:128, ts(kt, 128)],
                        rhs=qt_sb[b][64:128, ts(qs, QS)],
                        start=True, stop=True,
                    )
                    e = expp.tile([128, 2 * QS], BF16, tag="expT", name="e_t")
                    nc.scalar.activation(e, psS, mybir.ActivationFunctionType.Exp)
                    nc.tensor.matmul(
                        psC0,
                        lhsT=vaug[b][:, kt, 0:66],
                        rhs=e[:, 0:QS],
                        start=(kt == 0),
                        stop=(kt == NKT - 1),
                    )
                    nc.tensor.matmul(
                        psC1,
                        lhsT=vaug[b][:, kt, 66:VW],
                        rhs=e[:, QS:2 * QS],
                        start=(kt == 0),
                        stop=(kt == NKT - 1),
                    )

                # evacuate ctx + Z rows from PSUM immediately.
                csb0 = csbp.tile([64, QS], F32, tag="csb", name="csb0")
                csb1 = csbp.tile([64, QS], F32, tag="csb", name="csb1")
                nc.vector.tensor_copy(out=csb0, in_=psC0[0:64, :])
                nc.vector.tensor_copy(out=csb1, in_=psC1[0:64, :])
                # zsb rows (base partition 64): 64 <- Z1, 65 <- Z0.
                # Copy psC0[64:66] first (zero row + Z0), then overwrite
                # row 64 with Z1 from psC1.
                zsb = zpool.tile([66, QS], F32, tag="zsb", name="zsb")
                nc.vector.tensor_copy(out=zsb[64:66, :], in_=psC0[64:66, :])
                nc.vector.tensor_copy(out=zsb[64:65, :], in_=psC1[64:65, :])
                zr = zpool.tile([66, QS], F32, tag="zrec", name="zrec")
                nc.vector.reciprocal(out=zr[64:66, :], in_=zsb[64:66, :])

                # partition-broadcast 1/Z via DRAM roundtrip.
                zrow = zscr[b * NQS + qs, :]
                nc.sync.dma_start(
                    out=bass.AP(tensor=zrow.tensor, offset=zrow.offset,
                                ap=[[QS, 2], [1, QS]]),
                    in_=zr[64:66, :])
                bc0 = bcpool.tile([64, QS], F32, tag="bc", name="bc0")
                bc1 = bcpool.tile([64, QS], F32, tag="bc", name="bc1")
                # zscr row: [0:QS] = 1/Z1 (from partition 64), [QS:2QS] = 1/Z0
                for h, bc in ((1, bc0), (0, bc1)):
                    seg = zscr[b * NQS + qs, ds(h * QS, QS)]
                    nc.sync.dma_start(
                        out=bc,
                        in_=bass.AP(tensor=seg.tensor, offset=seg.offset,
                                    ap=[[0, 64]] + list(seg.ap)))
                stack_t = stackp.tile([128, QS], BF16, tag="stack")
                hb = hbufp.tile([64, QS], BF16, tag="hbuf")
                nc.vector.tensor_mul(stack_t[0:64, :], csb0, bc0)
                nc.vector.tensor_mul(hb, csb1, bc1)
                nc.sync.dma_start(out=stack_t[64:128, :], in_=hb)

                if pending is not None:
                    emit_wo(*pending)
                pending = (stack_t, b, qs)

        emit_wo(*pending)

    nc.compile()
    return nc


_NC = None


def _get_nc():
    global _NC
    if _NC is None:
        _NC = _build_program()
    return _NC


def _host_prep(query, key, value, Wq, bq, Wk, bk, Wv, bv, Wo, bo):
    bf16 = ml_dtypes.bfloat16
    f32 = np.float32
    q = np.asarray(query, f32)
    k = np.asarray(key, f32)
    v = np.asarray(value, f32)
    Wq = np.asarray(Wq, f32)
    Wk = np.asarray(Wk, f32)
    Wv = np.asarray(Wv, f32)
    Wo = np.asarray(Wo, f32)
    bq = np.asarray(bq, f32)
    bk = np.asarray(bk, f32)
    bv = np.asarray(bv, f32)

    scale = 1.0 / np.sqrt(DH).astype(f32)
    xqT = np.ascontiguousarray(q.transpose(0, 2, 1)).astype(bf16)
    xkT = np.ascontiguousarray(k.transpose(0, 2, 1)).astype(bf16)
    xvT = np.ascontiguousarray(v.transpose(0, 2, 1)).astype(bf16)

    in_maps = []
    for c in range(NCORES):
        sl = slice(c * DLOC, (c + 1) * DLOC)
        in_maps.append({
            "xqT": xqT, "xkT": xkT, "xvT": xvT,
            "wq": np.ascontiguousarray(Wq[:, sl] * scale).astype(bf16),
            "wk": np.ascontiguousarray(Wk[:, sl]).astype(bf16),
            "wv": np.ascontiguousarray(Wv[:, sl]).astype(bf16),
            "bq": np.ascontiguousarray((bq[sl] * scale).reshape(DLOC, 1)),
            "bk": np.ascontiguousarray(bk[sl].reshape(DLOC, 1)),
            "bv": np.ascontiguousarray(bv[sl].reshape(DLOC, 1)),
            "wo": np.ascontiguousarray(Wo[sl, :]).astype(bf16),
        })
    return in_maps


def _run(in_maps, trace=False):
    nc = _get_nc()
    return run_bass_kernel_spmd(nc, in_maps, list(range(NCORES)), trace=trace)


def kernel(query, key, value, Wq, bq, Wk, bk, Wv, bv, Wo, bo):
    in_maps = _host_prep(query, key, value, Wq, bq, Wk, bk, Wv, bv, Wo, bo)
    res = _run(in_maps)
    acc = np.zeros((B * N, D), np.float32)
    for c in range(NCORES):
        acc += np.asarray(res.results[c]["outp"], np.float32)
    acc += np.asarray(bo, np.float32)[None, :]
    return acc.reshape(B, N, D)


# revision 15
# speedup vs baseline: 1.4992x; 1.0505x over previous
"""Multi-head attention (B=2, N=2048, D=1024, H=16, dh=64) on 8 TRN2 cores.

Sharding: tensor-parallel over heads -- 2 heads per core. Each core computes
its heads' Q/K/V projections, attention, and a partial output projection
(rows of Wo for its heads); the host sums the 8 partial outputs (bf16).

Per-core layout strategy (v2 -- engine-balanced, HAM-warm):
  - Projections contract D with W stationary, producing Q^T/K^T/V^T
    ([local_dim, tok]); bias-adds run on the Scalar engine (idle then).
  - Scores: per k-tile, the two heads' [128,512] score matmuls are
    row-tiled (K=64 at array rows 0-63 / 64-127) into one [128,1024]
    two-bank PSUM tile; ONE exp activation covers both heads.
  - V is transposed via PE and packed into one vaug tile per batch:
    [V0 | 1 | V1 | 0 | 1] so head0's ctx matmul (M=65) puts Z0 at PSUM
    partition 64 and head1's (M=66) puts Z1 at partition 65 -- the two
    denominator rows land on distinct partitions and need only ONE
    reciprocal per q-slice.
  - ctx rows are evacuated to SBUF immediately (frees PSUM), normalized
    by a DRAM-roundtrip broadcast of 1/Z, and fed to the Wo matmul.
  - Partial outputs are written bf16 (halves writeback DMA).
"""

import numpy as np
import ml_dtypes
from contextlib import ExitStack

import concourse.bass as bass
import concourse.tile as tile
from concourse import bacc, mybir
from concourse.bass import ts, ds
from concourse.bass_utils import run_bass_kernel_spmd
from concourse.masks import make_identity

BF16 = mybir.dt.bfloat16
F32 = mybir.dt.float32

B = 2
N = 2048          # tokens per batch
D = 1024          # model dim
NCORES = 8
DLOC = 128        # local dims per core (2 heads x 64)
DH = 64
QS = 512          # q slice
NQS = N // QS     # 4 per batch
NKT = N // 128    # 16 k-tiles of 128
NDCH = D // 128   # 8 d-chunks
VW = 131          # vaug width: V0(64) | zero | ones | V1(64) | ones


def _build_program():
    nc = bacc.Bacc("TRN2", target_bir_lowering=False, debug=False)

    xT = {}
    w = {}
    bias = {}
    for t in ("q", "k", "v"):
        xT[t] = nc.dram_tensor(f"x{t}T", [B, D, N], BF16, kind="ExternalInput").ap()
        w[t] = nc.dram_tensor(f"w{t}", [D, DLOC], BF16, kind="ExternalInput").ap()
        bias[t] = nc.dram_tensor(f"b{t}", [DLOC, 1], F32, kind="ExternalInput").ap()
    wo = nc.dram_tensor("wo", [DLOC, D], BF16, kind="ExternalInput").ap()
    outp = nc.dram_tensor("outp", [B * N, D], BF16, kind="ExternalOutput").ap()
    zscr = nc.dram_tensor("zscr", [B * NQS, 2 * QS], F32).ap()

    with ExitStack() as ctx:
        tc = ctx.enter_context(tile.TileContext(nc))

        const = ctx.enter_context(tc.tile_pool(name="const", bufs=1))
        xpool = ctx.enter_context(tc.tile_pool(name="xchunks", bufs=16))
        qkpool = ctx.enter_context(tc.tile_pool(name="qk", bufs=4))
        vtpool = ctx.enter_context(tc.tile_pool(name="vt", bufs=2))
        vaugp = ctx.enter_context(tc.tile_pool(name="vaug", bufs=2))
        expp = ctx.enter_context(tc.tile_pool(name="expT", bufs=3))
        zpool = ctx.enter_context(tc.tile_pool(name="zr", bufs=2))
        bcpool = ctx.enter_context(tc.tile_pool(name="bc", bufs=4))
        csbp = ctx.enter_context(tc.tile_pool(name="csb", bufs=4))
        stackp = ctx.enter_context(tc.tile_pool(name="stack", bufs=2))
        hbufp = ctx.enter_context(tc.tile_pool(name="hbuf", bufs=2))
        outsb = ctx.enter_context(tc.tile_pool(name="outsb", bufs=3))

        pp_shared = ctx.enter_context(tc.tile_pool(name="pp_shared", bufs=2, space="PSUM"))
        pp_sc = ctx.enter_context(tc.tile_pool(name="pp_sc", bufs=2, space="PSUM"))
        pp_ctx = ctx.enter_context(tc.tile_pool(name="pp_ctx", bufs=2, space="PSUM"))

        # ---- constants ----
        ident = const.tile([128, 128], BF16, tag="ident")
        make_identity(nc, ident)
        # PE warmup: keep TensorE busy during the initial x-DMA wait so the
        # HAM clock gate reaches full rate before the projections start.
        warm = pp_shared.tile([128, 128], F32, tag="shared", name="warm")
        NWARM = 160
        for i in range(NWARM):
            nc.tensor.matmul(warm, lhsT=ident, rhs=ident,
                             start=(i == 0), stop=(i == NWARM - 1))
        w_sb = {}
        b_sb = {}
        for t in ("q", "k", "v"):
            w_sb[t] = const.tile([128, NDCH, DLOC], BF16, tag=f"w{t}", name=f"w{t}sb")
            nc.sync.dma_start(out=w_sb[t], in_=w[t].rearrange("(c p) m -> p c m", p=128))
            b_sb[t] = const.tile([128, 1], F32, tag=f"b{t}", name=f"b{t}sb")
            nc.sync.dma_start(out=b_sb[t], in_=bias[t])
        wo_sb = const.tile([128, D], BF16, tag="wo")
        nc.sync.dma_start(out=wo_sb, in_=wo)

        # ---- projections ----
        qt_sb = {}   # [b] -> [128, N] bf16  (Q^T, local dims on partitions)
        kt_sb = {}
        vaug = {}    # [b] -> [128, NKT, VW] bf16

        for b in range(B):
            xtiles = {}
            for t in ("q", "k", "v"):
                for c in range(NDCH):
                    xt_ = xpool.tile([128, N], BF16, tag="x", name="xt")
                    nc.sync.dma_start(out=xt_, in_=xT[t][b, ts(c, 128), :])
                    xtiles[(t, c)] = xt_

            for t in ("q", "k", "v"):
                if t == "v":
                    tgt = vtpool.tile([128, N], BF16, tag="vt", name="vt_t")
                else:
                    tgt = qkpool.tile([128, N], BF16, tag="qk", name="qk_t")
                for s in range(NQS):
                    ps = pp_shared.tile([128, QS], F32, tag="shared", name="ps_proj")
                    for c in range(NDCH):
                        nc.tensor.matmul(
                            ps,
                            lhsT=w_sb[t][:, c, :],
                            rhs=xtiles[(t, c)][:, ts(s, QS)],
                            start=(c == 0),
                            stop=(c == NDCH - 1),
                        )
                    # bias-add: Scalar engine for b=0 (idle then); DVE for
                    # b=1 (its projections overlap b=0 attention, where the
                    # Scalar engine is busy with exp).
                    if b == 0:
                        nc.scalar.activation(
                            tgt[:, ts(s, QS)], ps,
                            mybir.ActivationFunctionType.Identity,
                            bias=b_sb[t],
                        )
                    else:
                        nc.vector.tensor_scalar_add(tgt[:, ts(s, QS)], ps, b_sb[t])
                if t == "q":
                    qt_sb[b] = tgt
                elif t == "k":
                    kt_sb[b] = tgt
                else:
                    # transpose V^T -> V (tokens on partitions), pack both
                    # heads + denominator columns into one vaug tile.
                    va = vaugp.tile([128, NKT, VW], BF16, tag="vaug", name="va_t")
                    nc.vector.memset(va[:, :, 64:65], 0.0)
                    nc.vector.memset(va[:, :, 65:66], 1.0)
                    nc.vector.memset(va[:, :, 130:131], 1.0)
                    vaug[b] = va
                    for tk in range(NKT):
                        pt = pp_shared.tile([128, 128], BF16, tag="shared", name="pt_tr")
                        nc.tensor.transpose(pt, tgt[:, ts(tk, 128)], ident)
                        # one strided copy: head h -> cols [66h, 66h+64)
                        dst = bass.AP(
                            tensor=va.tensor,
                            offset=va.offset + tk * VW,
                            ap=[list(va.ap[0]), [66, 2], [1, 64]],
                        )
                        src = bass.AP(
                            tensor=pt.tensor,
                            offset=pt.offset,
                            ap=[list(pt.ap[0]), [64, 2], [1, 64]],
                        )
                        nc.vector.tensor_copy(out=dst, in_=src)

        # ---- attention (wo-phase deferred by one q-slice for overlap) ----
        pending = None  # (stack_tile, b, qs)

        def emit_wo(stack_t, b_, qs_):
            for qsub in range(QS // 128):
                ob = outsb.tile([128, D], BF16, tag="out", name="ob_out")
                for od in range(D // QS):
                    pw = pp_shared.tile([128, QS], F32, tag="shared", name="pw_wo")
                    nc.tensor.matmul(
                        pw,
                        lhsT=stack_t[:, ts(qsub, 128)],
                        rhs=wo_sb[:, ts(od, QS)],
                        start=True, stop=True,
                    )
                    nc.vector.tensor_copy(out=ob[:, ts(od, QS)], in_=pw)
                row0 = b_ * N + qs_ * QS + qsub * 128
                nc.sync.dma_start(out=outp[ds(row0, 128), :], in_=ob)

        for b in range(B):
            for qs in range(NQS):
                # psC0 rows: 0-63 ctx0, 64 zero, 65 Z0; psC1: 0-63 ctx1, 64 Z1
                psC0 = pp_ctx.tile([66, QS], F32, tag="ctx", name="ps_ctx0")
                psC1 = pp_ctx.tile([65, QS], F32, tag="ctx", name="ps_ctx1")
                for kt in range(NKT):
                    psS = pp_sc.tile([128, 2 * QS], F32, tag="sc", name="ps_sc")
                    nc.tensor.matmul(
                        psS[:, 0:QS],
                        lhsT=kt_sb[b][0:64, ts(kt, 128)],
                        rhs=qt_sb[b][0:64, ts(qs, QS)],
                        start=True, stop=True,
                    )
                    nc.tensor.matmul(
                        psS[:, QS:2 * QS],
                        lhsT=kt_sb[b][64:128, ts(kt, 128)],
                        rhs=qt_sb[b][64:128, ts(qs, QS)],
                        start=True, stop=True,
                    )
                    e = expp.tile([128, 2 * QS], BF16, tag="expT", name="e_t")
                    nc.scalar.activation(e, psS, mybir.ActivationFunctionType.Exp)
                    nc.tensor.matmul(
                        psC0,
                        lhsT=vaug[b][:, kt, 0:66],
                        rhs=e[:, 0:QS],
                        start=(kt == 0),
                        stop=(kt == NKT - 1),
                    )
                    nc.tensor.matmul(
                        psC1,
                        lhsT=vaug[b][:, kt, 66:VW],
                        rhs=e[:, QS:2 * QS],
                        start=(kt == 0),
                        stop=(kt == NKT - 1),
                    )

                # evacuate full PSUM tiles in ONE copy each (frees psC slots
                # fastest); Z rows are then re-staged from SBUF off the
                # critical PSUM path.
                csb0 = csbp.tile([66, QS], F32, tag="csb", name="csb0")
                csb1 = csbp.tile([65, QS], F32, tag="csb", name="csb1")
                nc.vector.tensor_copy(out=csb0, in_=psC0)
                nc.vector.tensor_copy(out=csb1, in_=psC1)
                # zsb rows (base partition 64): 64 <- Z1, 65 <- Z0.
                zsb = zpool.tile([66, QS], F32, tag="zsb", name="zsb")
                nc.vector.tensor_copy(out=zsb[64:66, :], in_=csb0[64:66, :])
                nc.vector.tensor_copy(out=zsb[64:65, :], in_=csb1[64:65, :])
                zr = zpool.tile([66, QS], F32, tag="zrec", name="zrec")
                nc.vector.reciprocal(out=zr[64:66, :], in_=zsb[64:66, :])

                # partition-broadcast 1/Z via DRAM roundtrip.
                zrow = zscr[b * NQS + qs, :]
                nc.sync.dma_start(
                    out=bass.AP(tensor=zrow.tensor, offset=zrow.offset,
                                ap=[[QS, 2], [1, QS]]),
                    in_=zr[64:66, :])
                bc0 = bcpool.tile([64, QS], F32, tag="bc", name="bc0")
                bc1 = bcpool.tile([64, QS], F32, tag="bc", name="bc1")
                # zscr row: [0:QS] = 1/Z1 (from partition 64), [QS:2QS] = 1/Z0
                for h, bc in ((1, bc0), (0, bc1)):
                    seg = zscr[b * NQS + qs, ds(h * QS, QS)]
                    nc.sync.dma_start(
                        out=bc,
                        in_=bass.AP(tensor=seg.tensor, offset=seg.offset,
                                    ap=[[0, 64]] + list(seg.ap)))
                stack_t = stackp.tile([128, QS], BF16, tag="stack")
                hb = hbufp.tile([64, QS], BF16, tag="hbuf")
                nc.vector.tensor_mul(stack_t[0:64, :], csb0[0:64, :], bc0)
                nc.vector.tensor_mul(hb, csb1[0:64, :], bc1)
                nc.sync.dma_start(out=stack_t[64:128, :], in_=hb)

                if pending is not None:
                    emit_wo(*pending)
                pending = (stack_t, b, qs)

        emit_wo(*pending)

    nc.compile()
    return nc


_NC = None


def _get_nc():
    global _NC
    if _NC is None:
        _NC = _build_program()
    return _NC


def _host_prep(query, key, value, Wq, bq, Wk, bk, Wv, bv, Wo, bo):
    bf16 = ml_dtypes.bfloat16
    f32 = np.float32
    q = np.asarray(query, f32)
    k = np.asarray(key, f32)
    v = np.asarray(value, f32)
    Wq = np.asarray(Wq, f32)
    Wk = np.asarray(Wk, f32)
    Wv = np.asarray(Wv, f32)
    Wo = np.asarray(Wo, f32)
    bq = np.asarray(bq, f32)
    bk = np.asarray(bk, f32)
    bv = np.asarray(bv, f32)

    scale = 1.0 / np.sqrt(DH).astype(f32)
    xqT = np.ascontiguousarray(q.transpose(0, 2, 1)).astype(bf16)
    xkT = np.ascontiguousarray(k.transpose(0, 2, 1)).astype(bf16)
    xvT = np.ascontiguousarray(v.transpose(0, 2, 1)).astype(bf16)

    in_maps = []
    for c in range(NCORES):
        sl = slice(c * DLOC, (c + 1) * DLOC)
        in_maps.append({
            "xqT": xqT, "xkT": xkT, "xvT": xvT,
            "wq": np.ascontiguousarray(Wq[:, sl] * scale).astype(bf16),
            "wk": np.ascontiguousarray(Wk[:, sl]).astype(bf16),
            "wv": np.ascontiguousarray(Wv[:, sl]).astype(bf16),
            "bq": np.ascontiguousarray((bq[sl] * scale).reshape(DLOC, 1)),
            "bk": np.ascontiguousarray(bk[sl].reshape(DLOC, 1)),
            "bv": np.ascontiguousarray(bv[sl].reshape(DLOC, 1)),
            "wo": np.ascontiguousarray(Wo[sl, :]).astype(bf16),
        })
    return in_maps


def _run(in_maps, trace=False):
    nc = _get_nc()
    return run_bass_kernel_spmd(nc, in_maps, list(range(NCORES)), trace=trace)


def kernel(query, key, value, Wq, bq, Wk, bk, Wv, bv, Wo, bo):
    in_maps = _host_prep(query, key, value, Wq, bq, Wk, bk, Wv, bv, Wo, bo)
    res = _run(in_maps)
    acc = np.zeros((B * N, D), np.float32)
    for c in range(NCORES):
        acc += np.asarray(res.results[c]["outp"], np.float32)
    acc += np.asarray(bo, np.float32)[None, :]
    return acc.reshape(B, N, D)
